# revision 15
# baseline (speedup 1.0000x reference)
"""DCRNN diffusion-conv GRU cell (single step, zero initial hidden state) on
8 Trainium2 NeuronCores.

Math (with H0 = 0 the reference cell reduces exactly to):
    out[b] = sigmoid(-(pre_z)) * tanh(pre_h)
    pre_z  = X Wz00 + Mo Wz01 + Mi Wz11 + bz      (Wg00 = (Wg[0,0]+Wg[1,0])[:128])
    pre_h  = X Wh00 + Mo Wh01 + Mi Wh11 + bh
    Mo[n]  = sum_{e: dst_e = n} coef_o[e] * X[src_e]
    Mi[n]  = sum_{e: src_e = n} coef_i[e] * X[dst_e]
(R / Wr / br are dead: H0*R = 0 and Xc2 == Xc.)

Sharding: nodes are padded to 5120 and cut into 40 chunks of 128; core g owns
chunks [5g, 5g+5) for ALL 4 batches.  Rows of x for all batches are packed
into 512-float gather rows, so one DMA-gather descriptor serves 4 batches and
one per-edge-tile one-hot matrix serves 4 batches.  The scatter-add runs on
the tensor engine: for each edge tile, PSUM[f, n] += M_tile[e, f].T @ O[e, n]
where O[e, n] = coef[e] * (n == dst_local[e]) is built by one DVE
tensor_scalar (is_equal then mult) from an iota row.
"""
import math
import os

import numpy as np

import concourse.bacc as bacc
import concourse.tile as tile
from concourse import mybir
from concourse.bass_utils import run_bass_kernel_spmd

P = 128           # partitions / chunk width / feature width
N_CORES = 8


def _prep(x, edge_index, edge_weight):
    """Host-side index/layout prep. Returns per-core input maps (minus the
    shared tensors) and metadata."""
    B, N, F = x.shape
    assert F == P
    E = edge_index.shape[1]
    n_chunks_total = math.ceil(N / P / N_CORES) * N_CORES  # 40
    npad = n_chunks_total * P                              # 5120
    cpc = n_chunks_total // N_CORES                        # chunks per core
    src = edge_index[0].astype(np.int64)
    dst = edge_index[1].astype(np.int64)
    ew = edge_weight.astype(np.float32)

    deg_out = np.bincount(src, weights=ew.astype(np.float64), minlength=N)
    deg_in = np.bincount(dst, weights=ew.astype(np.float64), minlength=N)
    with np.errstate(divide="ignore"):
        dinv_out = np.where(deg_out > 0, 1.0 / deg_out, 0.0).astype(np.float32)
        dinv_in = np.where(deg_in > 0, 1.0 / deg_in, 0.0).astype(np.float32)
    coef_o = ew * dinv_out[src]
    coef_i = ew * dinv_in[dst]

    # dirs: 0 = "o" (scatter to dst, gather src), 1 = "i" (scatter to src, gather dst)
    dirs = [(dst, src, coef_o), (src, dst, coef_i)]

    # group edges by (dir, scatter-chunk)
    grouped = []       # grouped[d][gc] = (gather_idx arr, dstloc arr, coef arr)
    max_cnt = 1
    for d, (sc, ga, cf) in enumerate(dirs):
        chunk_of = sc // P
        order = np.argsort(chunk_of, kind="stable")
        sc_s, ga_s, cf_s = sc[order], ga[order], cf[order]
        cnt = np.bincount(chunk_of, minlength=n_chunks_total)
        max_cnt = max(max_cnt, int(cnt.max()))
        offs = np.concatenate([[0], np.cumsum(cnt)])
        per_chunk = []
        for gc in range(n_chunks_total):
            s, e = offs[gc], offs[gc + 1]
            per_chunk.append((ga_s[s:e], sc_s[s:e] - gc * P, cf_s[s:e]))
        grouped.append(per_chunk)

    t_max = math.ceil(max_cnt / P)
    e_cap = t_max * P

    # packed gather source: xp[n, b*P + f] = x[b, n, f]
    xp = np.zeros((npad, B * P), np.float32)
    xp[:N] = np.asarray(x).transpose(1, 0, 2).reshape(N, B * P)

    xpad = np.zeros((B, npad, P), np.float32)
    xpad[:, :N] = x

    gc_cols = 2 * cpc * t_max * 8
    sc_cols = 2 * cpc * t_max * 2
    per_core = []
    for g in range(N_CORES):
        gidx = np.zeros((P, gc_cols), np.int16)
        scal = np.zeros((P, sc_cols), np.float32)
        for d in range(2):
            for lc in range(cpc):
                gc = g * cpc + lc
                ga, dl, cf = grouped[d][gc]
                cnt = len(ga)
                gaf = np.zeros(e_cap, np.int16)
                gaf[:cnt] = ga
                dlf = np.zeros(e_cap, np.float32)
                dlf[:cnt] = dl
                cff = np.zeros(e_cap, np.float32)
                cff[:cnt] = cf
                # gather idx layout: idx i -> [i % 16, base + i // 16],
                # replicated over the 8 groups of 16 partitions
                blk = gaf.reshape(t_max * 8, 16).T  # [16, t_max*8]
                base = (d * cpc + lc) * t_max * 8
                gidx[:, base:base + t_max * 8] = np.tile(blk, (8, 1))
                sbase = ((d * cpc + lc) * t_max) * 2
                scal[:, sbase + 0:sbase + 2 * t_max:2] = dlf.reshape(t_max, P).T
                scal[:, sbase + 1:sbase + 2 * t_max:2] = cff.reshape(t_max, P).T
        # dense-phase transposed x for this core's nodes:
        # xT[k, b*(cpc*P) + j] = x[b, g*cpc*P + j, k]
        xs = xpad[:, g * cpc * P:(g + 1) * cpc * P, :]        # [B, cpc*P, P]
        xT = np.ascontiguousarray(
            xs.transpose(2, 0, 1).reshape(P, B * cpc * P))
        per_core.append({"gidx": gidx, "scal": scal, "xT": xT})

    meta = dict(B=B, N=N, npad=npad, cpc=cpc, t_max=t_max,
                gc_cols=gc_cols, sc_cols=sc_cols)
    return xp, per_core, meta


def _build(meta):
    B = meta["B"]
    npad = meta["npad"]
    cpc = meta["cpc"]
    t_max = meta["t_max"]
    npc = cpc * P            # nodes per core
    ycols = B * npc
    elem = B * P             # packed gather row width (floats)

    nc = bacc.Bacc("TRN2", target_bir_lowering=False, debug=False,
                   num_devices=N_CORES)
    f32 = mybir.dt.float32
    xp_d = nc.dram_tensor("xp", [npad, elem], f32, kind="ExternalInput")
    xT_d = nc.dram_tensor("xT", [P, ycols], f32, kind="ExternalInput")
    gidx_d = nc.dram_tensor("gidx", [P, meta["gc_cols"]], mybir.dt.int16,
                            kind="ExternalInput")
    scal_d = nc.dram_tensor("scal", [P, meta["sc_cols"]], f32,
                            kind="ExternalInput")
    wt_d = nc.dram_tensor("wt", [P, 6 * P], f32, kind="ExternalInput")
    bias_d = nc.dram_tensor("bias", [P, 2], f32, kind="ExternalInput")
    iota_d = nc.dram_tensor("iota", [P, P], f32, kind="ExternalInput")
    yT_d = nc.dram_tensor("yT", [P, ycols], f32, kind="ExternalOutput")

    with tile.TileContext(nc) as tc:
        with (
            tc.tile_pool(name="const", bufs=1) as cpool,
            tc.tile_pool(name="gath", bufs=2) as gpool,
            tc.tile_pool(name="oh", bufs=4) as ohpool,
            tc.tile_pool(name="act", bufs=4) as apool,
            tc.tile_pool(name="psum", bufs=2, space="PSUM") as ppool,
        ):
            xT_s = cpool.tile([P, ycols], f32)
            nc.sync.dma_start(out=xT_s[:], in_=xT_d[:])
            gidx_s = cpool.tile([P, meta["gc_cols"]], mybir.dt.int16)
            nc.sync.dma_start(out=gidx_s[:], in_=gidx_d[:])
            scal_s = cpool.tile([P, meta["sc_cols"]], f32)
            nc.sync.dma_start(out=scal_s[:], in_=scal_d[:])
            wt_s = cpool.tile([P, 6 * P], f32)
            nc.sync.dma_start(out=wt_s[:], in_=wt_d[:])
            bias_s = cpool.tile([P, 2], f32)
            nc.sync.dma_start(out=bias_s[:], in_=bias_d[:])
            iota_s = cpool.tile([P, P], f32)
            nc.sync.dma_start(out=iota_s[:], in_=iota_d[:])

            m_s = [cpool.tile([P, ycols], f32, name=f"m{d}_s") for d in range(2)]
            y_s = cpool.tile([P, ycols], f32)

            # ---- sparse phase: Mo^T / Mi^T chunks via gather + one-hot matmul
            dbg_dirs = int(os.environ.get("K_DIRS", "2"))
            dbg_cpc = int(os.environ.get("K_CPC", str(cpc)))
            dbg_dense = int(os.environ.get("K_DENSE", "1"))
            if (dbg_dirs < 2 or dbg_cpc < cpc) and dbg_dense:
                for d in range(2):
                    nc.vector.memset(m_s[d][:], 0.0)
            if not dbg_dense:
                nc.vector.memset(y_s[:], 0.0)
                for d in range(2):
                    nc.vector.memset(m_s[d][:], 0.0)
                # route sparse results into y_s so they're observable
                m_s = [y_s, y_s]
            for d in range(dbg_dirs):
                for lc in range(dbg_cpc):
                    g_tile = gpool.tile([P, t_max * elem], f32, tag="g")
                    base = (d * cpc + lc) * t_max * 8
                    # the SWDGE descriptor ring holds 128 entries per DMA
                    # engine; >~2048 idxs in one gather overflows it and hangs
                    # the Q7, so split into <=1024-idx gathers (64/engine)
                    for t0 in range(0, t_max, 8):
                        t1 = min(t_max, t0 + 8)
                        nc.gpsimd.dma_gather(
                            out_ap=g_tile[:, t0 * elem:t1 * elem].rearrange(
                                "p (t e) -> p t e", e=elem),
                            in_ap=xp_d[:],
                            idxs_ap=gidx_s[:, base + t0 * 8:base + t1 * 8],
                            num_idxs=(t1 - t0) * P,
                            num_idxs_reg=(t1 - t0) * P,
                            elem_size=elem,
                        )
                    pm = [
                        ppool.tile([P, P], dtype=f32, name=f"pm{b}", tag=f"pm{b}",
                                   bufs=1)
                        for b in range(B)
                    ]
                    sbase = ((d * cpc + lc) * t_max) * 2
                    for t in range(t_max):
                        oh = ohpool.tile([P, P], f32, tag="oh")
                        nc.vector.tensor_scalar(
                            out=oh[:],
                            in0=iota_s[:],
                            scalar1=scal_s[:, sbase + 2 * t:sbase + 2 * t + 1],
                            scalar2=scal_s[:, sbase + 2 * t + 1:sbase + 2 * t + 2],
                            op0=mybir.AluOpType.is_equal,
                            op1=mybir.AluOpType.mult,
                        )
                        for b in range(B):
                            nc.tensor.matmul(
                                out=pm[b][:],
                                lhsT=g_tile[:, t * elem + b * P:t * elem + (b + 1) * P],
                                rhs=oh[:],
                                start=(t == 0),
                                stop=(t == t_max - 1),
                            )
                    for b in range(B):
                        nc.vector.tensor_copy(
                            out=m_s[d][:, b * npc + lc * P:b * npc + (lc + 1) * P],
                            in_=pm[b][:],
                        )

            # ---- dense phase
            for lc in range(cpc if dbg_dense else 0):
                for b in range(B):
                    col = b * npc + lc * P
                    psz = ppool.tile([P, P], dtype=f32, name="psz", tag="psz",
                                     bufs=2)
                    psh = ppool.tile([P, P], dtype=f32, name="psh", tag="psh",
                                     bufs=2)
                    for gi, w0 in enumerate((0, 3)):
                        pt = (psz if gi == 0 else psh)[:]
                        nc.tensor.matmul(
                            out=pt, lhsT=wt_s[:, w0 * P:(w0 + 1) * P],
                            rhs=xT_s[:, col:col + P], start=True, stop=False)
                        nc.tensor.matmul(
                            out=pt, lhsT=wt_s[:, (w0 + 1) * P:(w0 + 2) * P],
                            rhs=m_s[0][:, col:col + P], start=False, stop=False)
                        nc.tensor.matmul(
                            out=pt, lhsT=wt_s[:, (w0 + 2) * P:(w0 + 3) * P],
                            rhs=m_s[1][:, col:col + P], start=False, stop=True)
                    za = apool.tile([P, P], f32, tag="za")
                    nc.scalar.activation(
                        out=za[:], in_=psz[:],
                        func=mybir.ActivationFunctionType.Sigmoid,
                        bias=bias_s[:, 0:1], scale=-1.0)
                    ha = apool.tile([P, P], f32, tag="ha")
                    nc.scalar.activation(
                        out=ha[:], in_=psh[:],
                        func=mybir.ActivationFunctionType.Tanh,
                        bias=bias_s[:, 1:2], scale=1.0)
                    nc.vector.tensor_tensor(
                        out=y_s[:, col:col + P], in0=za[:], in1=ha[:],
                        op=mybir.AluOpType.mult)

            nc.sync.dma_start(out=yT_d[:], in_=y_s[:])
    nc.compile()
    return nc


def _shared_inputs(Wz, bz, Wh, bh):
    wt = np.concatenate([
        (Wz[0, 0][:P] + Wz[1, 0][:P]), Wz[0, 1][:P], Wz[1, 1][:P],
        (Wh[0, 0][:P] + Wh[1, 0][:P]), Wh[0, 1][:P], Wh[1, 1][:P],
    ], axis=1).astype(np.float32)
    bias = np.stack([-bz, bh], axis=1).astype(np.float32)
    iota = np.tile(np.arange(P, dtype=np.float32)[None, :], (P, 1))
    return wt, bias, iota


def build_all(inputs):
    """Returns (nc, in_maps, meta). Split out so test.py can reuse."""
    x = np.asarray(inputs["x"], np.float32)
    edge_index = np.asarray(inputs["edge_index"])
    edge_weight = np.asarray(inputs["edge_weight"], np.float32)
    Wz = np.asarray(inputs["Wz"], np.float32)
    bz = np.asarray(inputs["bz"], np.float32)
    Wh = np.asarray(inputs["Wh"], np.float32)
    bh = np.asarray(inputs["bh"], np.float32)

    xp, per_core, meta = _prep(x, edge_index, edge_weight)
    wt, bias, iota = _shared_inputs(Wz, bz, Wh, bh)
    in_maps = []
    for g in range(N_CORES):
        m = dict(per_core[g])
        m["xp"] = xp
        m["wt"] = wt
        m["bias"] = bias
        m["iota"] = iota
        in_maps.append(m)
    nc = _build(meta)
    return nc, in_maps, meta


def assemble_output(results, meta):
    B, N, npad, cpc = meta["B"], meta["N"], meta["npad"], meta["cpc"]
    npc = cpc * P
    out = np.empty((B, npad, P), np.float32)
    for g in range(N_CORES):
        blk = results[g]["yT"].reshape(P, B, npc).transpose(1, 2, 0)
        out[:, g * npc:(g + 1) * npc, :] = blk
    return np.ascontiguousarray(out[:, :N, :])


def kernel(**inputs) -> np.ndarray:
    nc, in_maps, meta = build_all(inputs)
    res = run_bass_kernel_spmd(nc, in_maps, list(range(N_CORES)))
    return assemble_output(res.results, meta)


# revision 16
# speedup vs baseline: 1.7650x; 1.7650x over previous
"""DCRNN diffusion-conv GRU cell (single step, zero initial hidden state) on
8 Trainium2 NeuronCores.

Math: with H0 = 0 the reference cell reduces exactly to
    out[b] = sigmoid(-(pre_z)) * tanh(pre_h)
    pre_z  = X Wz00 + Mo Wz01 + Mi Wz11 + bz      (Wg00 = (Wg[0,0]+Wg[1,0])[:128])
    pre_h  = X Wh00 + Mo Wh01 + Mi Wh11 + bh
    Mo = Ao^T X,  Ao[m, n] = sum_{e: src=m, dst=n} coef_o[e]
    Mi = Ai^T X,  Ai[m, n] = sum_{e: dst=m, src=n} coef_i[e]
(R / Wr / br are dead code: H0*R = 0 so Xc2 == Xc.)

Strategy (v2, gather-free): nodes padded to 5120 = 40 chunks of 128; core g
owns output nodes [g*640, (g+1)*640) for ALL 4 batches.  The sparse diffusion
ops run as block-dense matmuls on the tensor engine: the host scatters the
per-edge coefficients into dense bf16 blocks A[:, core_cols] (graph structure
only — batch independent), and the kernel contracts X^T A over all 40
m-chunks into PSUM.  This costs ~150x the nominal sparse FLOPs but runs at
PE line rate and completely avoids the SWDGE descriptor-generation floor
(~6.5 ns/edge on the Q7) and slow DVE one-hot builds that a DMA-gather
formulation pays.

Precision: sparse path bf16 (A, X rounded; fp32 PSUM accumulate).  The dense
X@W00 term uses a bf16 split-float product (Xh Wh + Xh Wl + Xl Wh) to keep
overall rel err ~2e-3.
"""
import math
import os

import numpy as np
import ml_dtypes

import concourse.bacc as bacc
import concourse.tile as tile
from concourse import mybir
from concourse.bass_utils import run_bass_kernel_spmd

P = 128
N_CORES = 8
BF16 = ml_dtypes.bfloat16


def _prep(x, edge_index, edge_weight):
    B, N, F = x.shape
    assert F == P
    n_chunks = math.ceil(N / P / N_CORES) * N_CORES      # 40
    npad = n_chunks * P                                  # 5120
    cpc = n_chunks // N_CORES                            # 5
    npc = cpc * P                                        # 640
    src = edge_index[0].astype(np.int64)
    dst = edge_index[1].astype(np.int64)
    ew = edge_weight.astype(np.float32)

    deg_out = np.bincount(src, weights=ew.astype(np.float64), minlength=N)
    deg_in = np.bincount(dst, weights=ew.astype(np.float64), minlength=N)
    with np.errstate(divide="ignore"):
        dinv_out = np.where(deg_out > 0, 1.0 / deg_out, 0.0).astype(np.float32)
        dinv_in = np.where(deg_in > 0, 1.0 / deg_in, 0.0).astype(np.float32)
    coef_o = ew * dinv_out[src]
    coef_i = ew * dinv_in[dst]

    Ao = np.zeros((npad, npad), np.float32)
    np.add.at(Ao, (src, dst), coef_o)
    Ai = np.zeros((npad, npad), np.float32)
    np.add.at(Ai, (dst, src), coef_i)

    xpad = np.zeros((B, npad, P), np.float32)
    xpad[:, :N] = x
    xb = xpad.astype(BF16).astype(np.float32)
    xlo = (xpad - xb).astype(BF16)

    # lhsT layout for the sparse phase: xq[p, mb*B*P + b*P + f] = x[b, mb*P+p, f]
    xq = np.ascontiguousarray(
        xpad.reshape(B, n_chunks, P, P).transpose(2, 1, 0, 3)
        .reshape(P, n_chunks * B * P)).astype(BF16)

    per_core = []
    for g in range(N_CORES):
        cols = slice(g * npc, (g + 1) * npc)
        ao = np.ascontiguousarray(Ao[:, cols]).astype(BF16)
        ai = np.ascontiguousarray(Ai[:, cols]).astype(BF16)
        # dense-phase rhs: xT[k, b*npc + j] = x[b, g*npc + j, k] (hi and lo parts)
        xs = xpad[:, cols, :]
        xT = np.ascontiguousarray(
            xs.transpose(2, 0, 1).reshape(P, B * npc)).astype(BF16)
        xsl = xlo[:, cols, :].astype(np.float32)
        xTlo = np.ascontiguousarray(
            xsl.transpose(2, 0, 1).reshape(P, B * npc)).astype(BF16)
        per_core.append({"ao": ao, "ai": ai, "xT": xT, "xTlo": xTlo})

    meta = dict(B=B, N=N, npad=npad, cpc=cpc, n_chunks=n_chunks)
    return xq, per_core, meta


def _shared_inputs(Wz, bz, Wh, bh):
    def split(w):
        hi = w.astype(BF16).astype(np.float32)
        return hi.astype(BF16), (w - hi).astype(BF16)

    Wz00h, Wz00l = split(Wz[0, 0][:P] + Wz[1, 0][:P])
    Wh00h, Wh00l = split(Wh[0, 0][:P] + Wh[1, 0][:P])
    wt = np.concatenate([
        Wz00h, Wz00l, Wz[0, 1][:P].astype(BF16), Wz[1, 1][:P].astype(BF16),
        Wh00h, Wh00l, Wh[0, 1][:P].astype(BF16), Wh[1, 1][:P].astype(BF16),
    ], axis=1)
    bias = np.stack([-bz, bh], axis=1).astype(np.float32)
    return wt, bias


def _build(meta):
    B = meta["B"]
    npad = meta["npad"]
    cpc = meta["cpc"]
    nmb = meta["n_chunks"]          # m-chunks in the contraction
    npc = cpc * P                   # output nodes per core
    ycols = B * npc
    bf = mybir.dt.bfloat16
    f32 = mybir.dt.float32
    # n-groups of the per-core output columns per PSUM bank (free dim <= 512)
    ngrps = [(0, min(512, npc))]
    if npc > 512:
        ngrps.append((512, npc - 512))

    nc = bacc.Bacc("TRN2", target_bir_lowering=False, debug=False,
                   num_devices=N_CORES)
    ao_d = nc.dram_tensor("ao", [npad, npc], bf, kind="ExternalInput")
    ai_d = nc.dram_tensor("ai", [npad, npc], bf, kind="ExternalInput")
    xq_d = nc.dram_tensor("xq", [P, nmb * B * P], bf, kind="ExternalInput")
    xT_d = nc.dram_tensor("xT", [P, ycols], bf, kind="ExternalInput")
    xTlo_d = nc.dram_tensor("xTlo", [P, ycols], bf, kind="ExternalInput")
    wt_d = nc.dram_tensor("wt", [P, 8 * P], bf, kind="ExternalInput")
    bias_d = nc.dram_tensor("bias", [P, 2], f32, kind="ExternalInput")
    yT_d = nc.dram_tensor("yT", [P, ycols], f32, kind="ExternalOutput")

    with tile.TileContext(nc) as tc:
        with (
            tc.tile_pool(name="const", bufs=1) as cpool,
            tc.tile_pool(name="act", bufs=4) as apool,
            tc.tile_pool(name="psum", bufs=8, space="PSUM") as ppool,
        ):
            a_s = [cpool.tile([P, nmb * npc], bf, name=f"a{d}_s") for d in range(2)]
            # per-m-chunk loads so early matmuls unblock before the whole
            # matrix lands
            for mb in range(nmb):
                for d, src_d in enumerate((ao_d, ai_d)):
                    nc.sync.dma_start(
                        out=a_s[d][:, mb * npc:(mb + 1) * npc],
                        in_=src_d[mb * P:(mb + 1) * P, :])
            xq_s = cpool.tile([P, nmb * B * P], bf)
            nc.sync.dma_start(out=xq_s[:], in_=xq_d[:])
            xT_s = cpool.tile([P, ycols], bf)
            nc.sync.dma_start(out=xT_s[:], in_=xT_d[:])
            xTlo_s = cpool.tile([P, ycols], bf)
            nc.sync.dma_start(out=xTlo_s[:], in_=xTlo_d[:])
            wt_s = cpool.tile([P, 8 * P], bf)
            nc.sync.dma_start(out=wt_s[:], in_=wt_d[:])
            bias_s = cpool.tile([P, 2], f32)
            nc.sync.dma_start(out=bias_s[:], in_=bias_d[:])

            m_s = [cpool.tile([P, ycols], bf, name=f"m{d}_s") for d in range(2)]
            y_s = cpool.tile([P, ycols], f32)

            # ---- sparse phase: M_d^T[f, n] += sum_mb xq[mb,b]^T @ A_d[mb]
            for b in range(B):
                ps = {}
                for d in range(2):
                    for gi, (n0, nw) in enumerate(ngrps):
                        ps[(d, gi)] = ppool.tile(
                            [P, 512], dtype=f32, name=f"ps{d}{gi}", tag="ps")
                for mb in range(nmb):
                    lhsT = xq_s[:, (mb * B + b) * P:(mb * B + b + 1) * P]
                    for d in range(2):
                        for gi, (n0, nw) in enumerate(ngrps):
                            nc.tensor.matmul(
                                out=ps[(d, gi)][:, :nw],
                                lhsT=lhsT,
                                rhs=a_s[d][:, mb * npc + n0:mb * npc + n0 + nw],
                                start=(mb == 0),
                                stop=(mb == nmb - 1),
                            )
                for d in range(2):
                    for gi, (n0, nw) in enumerate(ngrps):
                        nc.vector.tensor_copy(
                            out=m_s[d][:, b * npc + n0:b * npc + n0 + nw],
                            in_=ps[(d, gi)][:, :nw])

            # ---- dense phase (per 128-node chunk, per batch)
            for lc in range(cpc):
                for b in range(B):
                    col = b * npc + lc * P
                    psz = ppool.tile([P, 512], dtype=f32, name="psz", tag="ps")
                    psh = ppool.tile([P, 512], dtype=f32, name="psh", tag="ps")
                    for gi, w0 in enumerate((0, 4)):
                        pt = (psz if gi == 0 else psh)[:, :P]
                        terms = [
                            (wt_s[:, (w0 + 0) * P:(w0 + 1) * P], xT_s),
                            (wt_s[:, (w0 + 1) * P:(w0 + 2) * P], xT_s),
                            (wt_s[:, (w0 + 0) * P:(w0 + 1) * P], xTlo_s),
                            (wt_s[:, (w0 + 2) * P:(w0 + 3) * P], m_s[0]),
                            (wt_s[:, (w0 + 3) * P:(w0 + 4) * P], m_s[1]),
                        ]
                        for ti, (w_ap, rhs_t) in enumerate(terms):
                            nc.tensor.matmul(
                                out=pt, lhsT=w_ap,
                                rhs=rhs_t[:, col:col + P],
                                start=(ti == 0), stop=(ti == len(terms) - 1))
                    za = apool.tile([P, P], f32, tag="za")
                    nc.scalar.activation(
                        out=za[:], in_=psz[:, :P],
                        func=mybir.ActivationFunctionType.Sigmoid,
                        bias=bias_s[:, 0:1], scale=-1.0)
                    ha = apool.tile([P, P], f32, tag="ha")
                    nc.scalar.activation(
                        out=ha[:], in_=psh[:, :P],
                        func=mybir.ActivationFunctionType.Tanh,
                        bias=bias_s[:, 1:2], scale=1.0)
                    nc.vector.tensor_tensor(
                        out=y_s[:, col:col + P], in0=za[:], in1=ha[:],
                        op=mybir.AluOpType.mult)

            nc.sync.dma_start(out=yT_d[:], in_=y_s[:])
    nc.compile()
    return nc


def build_all(inputs):
    """Returns (nc, in_maps, meta). Split out so test.py can reuse."""
    x = np.asarray(inputs["x"], np.float32)
    edge_index = np.asarray(inputs["edge_index"])
    edge_weight = np.asarray(inputs["edge_weight"], np.float32)
    Wz = np.asarray(inputs["Wz"], np.float32)
    bz = np.asarray(inputs["bz"], np.float32)
    Wh = np.asarray(inputs["Wh"], np.float32)
    bh = np.asarray(inputs["bh"], np.float32)

    xq, per_core, meta = _prep(x, edge_index, edge_weight)
    wt, bias = _shared_inputs(Wz, bz, Wh, bh)
    in_maps = []
    for g in range(N_CORES):
        m = dict(per_core[g])
        m["xq"] = xq
        m["wt"] = wt
        m["bias"] = bias
        in_maps.append(m)
    nc = _build(meta)
    return nc, in_maps, meta


def assemble_output(results, meta):
    B, N, npad, cpc = meta["B"], meta["N"], meta["npad"], meta["cpc"]
    npc = cpc * P
    out = np.empty((B, npad, P), np.float32)
    for g in range(N_CORES):
        blk = results[g]["yT"].reshape(P, B, npc).transpose(1, 2, 0)
        out[:, g * npc:(g + 1) * npc, :] = blk
    return np.ascontiguousarray(out[:, :N, :])


def kernel(**inputs) -> np.ndarray:
    nc, in_maps, meta = build_all(inputs)
    res = run_bass_kernel_spmd(nc, in_maps, list(range(N_CORES)))
    return assemble_output(res.results, meta)
